# revision 41
# baseline (speedup 1.0000x reference)
"""Multi-head attention kernel for Trainium2 (Bass/Tile), 8-core SPMD.

Problem: B=4, L=S=2048, H=8, E=D=64, fp32.
  scores = einsum('blhe,bshe->bhls', Q, K) * tau[b] + delta[b]
  A = softmax(scores / sqrt(E), axis=-1)
  out = einsum('bhls,bshd->blhd', A, V)

Key observations:
  - softmax(a*x + c) == softmax(a*x): the per-batch delta bias cancels.
  - attn_mask is all-False / unused by the reference.
  - B*H = 32 (b,h) pairs, each an independent L x S attention block.
    Shard 4 pairs per core across 8 cores; no cross-core comms.

Per-core kernel design (per (b,h) pair), all matmul operands fp16,
fp32 PSUM:
  - Scores are computed TRANSPOSED: ST[s, l] chunks of [128, 512] so that
    the PV matmul can consume exp(ST) directly as the moving operand with
    full K=128 contraction (no P transposes).
  - QK row-packed: two K=64 matmuls run concurrently in PE row groups
    (0,0)/(64,0); host supplies K^T chunk pairs split across partition
    halves and Q^T duplicated on both halves.
  - exp: 16 chunks/l-tile split 10 on ScalarE (activation Exp, per-batch
    tau/sqrt(E) folded into the activation scale, ~598ns/chunk incl the
    ~335ns fixed cost) + 6 on DVE (bit-trick, ~642ns/chunk), so the two
    exp engines run at ~6.0us/tile next to the PE's ~6.25us/tile.
    (Splitting ONE group's exp between SE and DVE halves to fine-tune
    the balance SLOWED every full DVE exp 1285->1520ns - concurrent
    SE+DVE reads of the same st pool's PSUM banks contend; do not
    retry.)
  - PV: lhsT = V' chunk [s=128, 128] with columns 0..63 all-ones and V
    in columns 64..127, rhs = exp chunk [128, 512], accumulated over 16
    s-chunks into O^T [128, 512] PSUM. The 64 ones columns replicate
    the softmax denominator across PSUM partitions 0..63 at zero PE
    cost. V at partition base 64 because >32-partition DVE accesses
    must start at partition 0 or 64 (verifier-enforced). (PSUM allows
    only ONE live accumulation chain per bank, so the l-sub-split
    "operand-swapped" PV variant with 4 interleaved chains in one bank
    silently corrupts - measured, do not retry.)
  - Tail, two stages: (1) reciprocal_approx_fast straight on PSUM
    partitions 0..63 (custom DVE ops need base_partition 0 - measured)
    - no partition broadcast needed thanks to the replicated
    denominator; (2) TWO units later, multiply O^T rows 64..127 out of
    PSUM (fp16 out) and store. The stage gap keeps the mul's
    cross-engine wait from head-of-line-blocking the next DVE exp,
    whose lookahead-QK WAR otherwise stalls the PE 1.3-2.2us per pair
    (measured).
  - Emission: units processed in SUPERSLOTS of two with LOOKAHEAD 3 and
    PV DELAY 2 - PE program order per superslot is [QK(i+3), QK(i+4),
    PV(i-2), PV(i-1)]. Adjacent same-mode matmuls halve the packed<->
    unpacked PE reconfigurations (~90ns exposed LDWEIGHTS each), and
    the PV delay gives the exp->PV RAW and the tail-mul ot WAR two
    unit-slots of slack. 8 uniform 2-chunk groups per l-tile cycle
    through 3 st pools keyed on the GLOBAL unit index mod 3 (strict
    rotation across tile boundaries). exp_pool MUST be 6-deep: with
    4 bufs, exp(i) WARs the delayed PV(i-4) and the whole pipeline
    slows ~0.6us/tile (measured).
  - Output is stored fp16 (DVE mul writes f16; host upcasts) - halves
    store traffic; adds ~2e-4 rel err vs the 2e-2 budget.

Host side only reshapes/slices/transposes/concats constant ones and
upcasts the fp16 output (sharding + layout); all math is on-device.
Measured: 148.4us (v0 baseline) -> 125.9us.
"""

import os
import numpy as np

B, L, S, H, E = 4, 2048, 2048, 8, 64
NCORES = 8
NP = (B * H) // NCORES  # pairs per core = 4

LT = 512          # l-tile size (columns of ST chunks / PV moving dim)
NLT = L // LT     # 4
NSC = S // 128    # 16 s-chunks
# 8 uniform groups of 2 s-chunks per l-tile, cycling through THREE
# 2-bank st PSUM pools with LOOKAHEAD 3 (trace showed the 2-pool /
# lookahead-2 scheme stalled the PE ~2us/tile on the QK(u+2) WAR
# against exp(u); a 3-deep rotation gives ~2.2us of pipeline slack vs
# the ~1.3us exp latency). stA+stB+stC (2 banks each) + O^T (2 bufs x
# 1 bank) = 8 PSUM banks.
GROUPS = (2,) * 8

_PROGRAM = None
LAST_RESULTS = None  # test harness reads exec_time_ns / trace path from here


def _build_program():
    import concourse.bass as bass
    import concourse.bacc as bacc
    import concourse.tile as tile
    from concourse import mybir

    f32 = mybir.dt.float32
    f16 = mybir.dt.float16
    EXP = mybir.ActivationFunctionType.Exp

    nc = bacc.Bacc("TRN2", target_bir_lowering=False, debug=False,
                   num_devices=NCORES)
    # qt2: Q^T duplicated on both partition halves [128, L].
    # kt2: K^T s-chunk pairs split across partition halves:
    #   kt2[0:64, j, :] = K^T chunk 2j, kt2[64:128, j, :] = chunk 2j+1.
    qT = nc.dram_tensor("qt", [NP, 2 * E, L], f32, kind="ExternalInput").ap()
    kT = nc.dram_tensor("kt", [NP, 2 * E, NSC // 2, 128], f32,
                        kind="ExternalInput").ap()
    v = nc.dram_tensor("v", [NP, S, E], f32, kind="ExternalInput").ap()
    taus = nc.dram_tensor("taus", [1, NP], f32, kind="ExternalInput").ap()
    o = nc.dram_tensor("o", [NP, E, L], f16, kind="ExternalOutput").ap()

    with tile.TileContext(nc) as tc:
        from contextlib import ExitStack
        with ExitStack() as ctx:
            consts = ctx.enter_context(tc.tile_pool(name="consts", bufs=1))
            kq_pool = ctx.enter_context(tc.tile_pool(name="kq", bufs=2))
            v_pool = ctx.enter_context(tc.tile_pool(name="vp", bufs=2))
            exp_pool = ctx.enter_context(tc.tile_pool(name="expp", bufs=7))
            tail_pool = ctx.enter_context(tc.tile_pool(name="tail", bufs=3))
            stA_pool = ctx.enter_context(
                tc.tile_pool(name="stA", bufs=1, space="PSUM"))
            stB_pool = ctx.enter_context(
                tc.tile_pool(name="stB", bufs=1, space="PSUM"))
            stC_pool = ctx.enter_context(
                tc.tile_pool(name="stC", bufs=1, space="PSUM"))
            ot_pool = ctx.enter_context(
                tc.tile_pool(name="ot", bufs=2, space="PSUM"))
            st_pools = (stA_pool, stB_pool, stC_pool)

            # tau[b] per pair, broadcast to all partitions; fold in 1/sqrt(E)
            tau_bc = consts.tile([128, NP], f32)
            nc.sync.dma_start(out=tau_bc, in_=taus.to_broadcast([128, NP]))
            a_all = consts.tile([128, NP], f32)
            nc.scalar.mul(a_all, tau_bc, 1.0 / np.sqrt(float(E)))
            # DVE bit-trick exp scale: tau/sqrt(E) * log2(e) * 2^10 (fp16
            # exponent scale). z = st*a2 + BIAS truncated to int16 and
            # bitcast to fp16 gives 2^(st*a1*log2e) with piecewise-linear
            # mantissa; BIAS folds the fp16 exponent offset (15<<10), +0.5
            # round-to-nearest, and a /1.0397 centering of the one-sided
            # interpolation error (keeps DVE chunks consistent with the
            # exact ScalarE chunks in the softmax mixture).
            a2_all = consts.tile([128, NP], f32)
            nc.scalar.mul(a2_all, tau_bc,
                          np.log2(np.e) * 1024.0 / np.sqrt(float(E)))

            # unit = one exp group: (pair, l-tile, group idx, size, chunk0)
            units = []
            for p in range(NP):
                for t in range(NLT):
                    c0 = 0
                    for gi, G in enumerate(GROUPS):
                        units.append((p, t, gi, G, c0))
                        c0 += G

            pair_tiles = {}
            deferred_casts = {}

            def emit_loads(p):
                # fp16 operands; gpsimd DMA casts f32 -> f16 in flight.
                # Split into first-use-first pieces so the first dependent
                # QK/PV can start long before the whole pair has landed.
                # Pair 0 is latency-critical (nothing hides it): its kt/qt
                # head chunks ride the TWO hardware DGE queues (sync +
                # scalar) as raw f32 in parallel with the gpsimd software
                # casting queue, then DVE (idle at startup) casts them.
                kt_sb = kq_pool.tile([128, NSC // 2, 128], f16, tag="kt")
                qt_sb = kq_pool.tile([128, L], f16, tag="qt")
                vp_sb = v_pool.tile([128, NSC, 128], f16, tag="vp")
                # V' = [ones | zeros x63 | V]: the ones column puts the
                # PV denominator row on PSUM partition 0, where
                # reciprocal_approx_fast can read it directly (custom
                # DVE ops need base_partition 0) - kills v0's 1-lane
                # denominator COPY. O^T rows land at partitions 64..127
                # so the tail's 64-partition DVE ops stay quadrant-
                # aligned (>32-wide DVE accesses must start at 0 or 64).
                # Ones fill ALL of V' columns 0..63 (GpSimd queue):
                # the PV matmul then replicates the softmax denominator
                # across PSUM partitions 0..63 at zero PE cost, so the
                # tail reciprocal runs directly on [64, LT] (same DVE
                # column count as [1, LT]) and the GpSimd
                # partition_broadcast round-trip disappears. For pair 0
                # the memset is emitted between the v-DMAs (gpsimd issue
                # slots are ~1us; the first v packets must start ASAP).
                vre = v[p].rearrange("(n q) e -> q n e", q=128)
                nc.gpsimd.memset(vp_sb[:, :, 0:64], 1.0)
                if p == 0:
                    # Pair 0 is latency-critical: spread it over all 3
                    # DMA queues (each ~45-67GB/s), f32-staged on the hw
                    # queues with DVE casts (DVE is idle until the first
                    # exp). qt tile 0 is host-DUPLICATED data: DMA only
                    # the unique [64, LT] half and replicate via two DVE
                    # casts - halves the first-QK critical DMA. kt24/
                    # kt48 land later, so their casts are emitted
                    # DEFERRED at superslot 2/4 tops (before that slot's
                    # exps - a cast emitted after an exp whose QK needs
                    # it would deadlock the DVE FIFO).
                    stage_kt = consts.tile([128, NSC // 2, 128], f32)
                    stage_qh = consts.tile([64, LT], f32)
                    nc.scalar.dma_start(out=stage_qh, in_=qT[p][0:64, 0:LT])
                    nc.sync.dma_start(out=stage_kt[:, 0:2, :],
                                      in_=kT[p][:, 0:2, :])
                    nc.scalar.dma_start(out=stage_kt[:, 4:8, :],
                                        in_=kT[p][:, 4:8, :])
                    nc.sync.dma_start(out=stage_kt[:, 2:4, :],
                                      in_=kT[p][:, 2:4, :])
                    nc.vector.tensor_copy(qt_sb[0:64, 0:LT], stage_qh)
                    nc.vector.tensor_copy(qt_sb[64:128, 0:LT], stage_qh)
                    nc.vector.tensor_copy(kt_sb[:, 0:2, :],
                                          stage_kt[:, 0:2, :])
                    # kt24 feeds QK(2) which is PRIMED, so its cast must
                    # be emitted here (before the prime). Only kt48's
                    # cast can defer - to superslot 0, emitted BEFORE
                    # the QK(4) append that reads it.
                    nc.vector.tensor_copy(kt_sb[:, 2:4, :],
                                          stage_kt[:, 2:4, :])
                    deferred_casts[0] = lambda: nc.vector.tensor_copy(
                        kt_sb[:, 4:8, :], stage_kt[:, 4:8, :])
                    nc.gpsimd.dma_start(out=vp_sb[:, 0:8, 64:64 + E],
                                        in_=vre[:, 0:8, :])
                    nc.gpsimd.dma_start(out=vp_sb[:, 8:16, 64:64 + E],
                                        in_=vre[:, 8:16, :])
                    # qt tile 1 gated tile 1's start (~21us on the
                    # serialized gpsimd queue): load only the unique
                    # [64, LT] half (halves its queue time) and
                    # replicate with one DVE f16 copy, deferred to
                    # superslot 4 - before the QK(8) append that reads
                    # it. qt2/qt3 have stride-slack; keep them full.
                    nc.gpsimd.dma_start(out=qt_sb[0:64, LT:2 * LT],
                                        in_=qT[p][0:64, LT:2 * LT])
                    deferred_casts[4] = lambda: nc.vector.tensor_copy(
                        qt_sb[64:128, LT:2 * LT], qt_sb[0:64, LT:2 * LT])
                    for t in range(2, NLT):
                        nc.gpsimd.dma_start(
                            out=qt_sb[:, t * LT:(t + 1) * LT],
                            in_=qT[p][:, t * LT:(t + 1) * LT])
                else:
                    nc.gpsimd.dma_start(out=kt_sb[:, 0:2, :],
                                        in_=kT[p][:, 0:2, :])
                    nc.gpsimd.dma_start(out=qt_sb[:, 0:LT],
                                        in_=qT[p][:, 0:LT])
                    nc.gpsimd.dma_start(out=kt_sb[:, 2:8, :],
                                        in_=kT[p][:, 2:8, :])
                    nc.gpsimd.dma_start(out=vp_sb[:, 0:8, 64:64 + E],
                                        in_=vre[:, 0:8, :])
                    for t in range(1, NLT):
                        nc.gpsimd.dma_start(
                            out=qt_sb[:, t * LT:(t + 1) * LT],
                            in_=qT[p][:, t * LT:(t + 1) * LT])
                    nc.gpsimd.dma_start(out=vp_sb[:, 8:16, 64:64 + E],
                                        in_=vre[:, 8:16, :])
                pair_tiles[p] = (kt_sb, qt_sb, vp_sb)

            def emit_qk(u, ui):
                p, t, gi, G, c0 = u
                kt_sb, qt_sb, _ = pair_tiles[p]
                st_ps = st_pools[ui % 3].tile([128, 2 * LT], f32)
                j = c0 // 2  # packed chunk-pair index
                nc.tensor.matmul(
                    st_ps[:, 0:LT],
                    lhsT=kt_sb[0:64, j, :],
                    rhs=qt_sb[0:64, t * LT:(t + 1) * LT],
                    start=True, stop=True, tile_position=(0, 0))
                nc.tensor.matmul(
                    st_ps[:, LT:2 * LT],
                    lhsT=kt_sb[64:128, j, :],
                    rhs=qt_sb[64:128, t * LT:(t + 1) * LT],
                    start=True, stop=True, tile_position=(64, 0))
                return st_ps

            cur_ot = [None]

            def emit_pv(u, ex):
                p, t, gi, G, c0 = u
                vp_sb = pair_tiles[p][2]
                if c0 == 0:
                    cur_ot[0] = ot_pool.tile([128, LT], f32, name="ot_ps",
                                             tag="ot_ps")
                for k in range(G):
                    c = c0 + k
                    nc.tensor.matmul(
                        cur_ot[0],
                        lhsT=vp_sb[:, c, :],
                        rhs=ex[:, k * LT:(k + 1) * LT],
                        start=(c == 0), stop=(c == NSC - 1))

            def emit_tail_recip(u, ot_tile):
                p, t, gi, G, c0 = u
                # Denominator sits replicated on PSUM partitions 0..63
                # (ones columns of V'): fast-recip straight from PSUM.
                rb = tail_pool.tile([64, LT], f32, tag="rb")
                nc.vector.reciprocal_approx_fast(out=rb,
                                                 in_=ot_tile[0:64, :])
                return rb

            def emit_tail_mul(u, ot_tile, rb, split=False):
                p, t, gi, G, c0 = u
                # Normalize reading O^T (rows 64..127) straight from
                # PSUM; split=True halves the chain across the two hw
                # DMA queues (used for the drain-critical final tiles).
                on = tail_pool.tile([64, LT], f16, tag="on")
                if split:
                    nc.vector.tensor_mul(on[:, 0:LT // 2],
                                         ot_tile[64:64 + E, 0:LT // 2],
                                         rb[:, 0:LT // 2])
                    nc.sync.dma_start(
                        out=o[p, :, t * LT:t * LT + LT // 2],
                        in_=on[:, 0:LT // 2])
                    nc.vector.tensor_mul(on[:, LT // 2:LT],
                                         ot_tile[64:64 + E, LT // 2:LT],
                                         rb[:, LT // 2:LT])
                    nc.scalar.dma_start(
                        out=o[p, :, t * LT + LT // 2:(t + 1) * LT],
                        in_=on[:, LT // 2:LT])
                else:
                    nc.vector.tensor_mul(on, ot_tile[64:64 + E, :], rb)
                    nc.sync.dma_start(out=o[p, :, t * LT:(t + 1) * LT],
                                      in_=on)

            # Software-pipelined emission, lookahead 3: the PE program order
            # per iteration is [QK(u+3), PV(u)], so when PV(u) blocks on
            # exp(u), QK(u+1..3) are already past it, and the WAR of
            # QK(u+3) on exp(u) (same st pool, 3-deep rotation) has ~3
            # group-slots of PE work to hide the exp latency.
            DVE_GROUPS = (1, 3, 5)  # groups exp'd on Vector (bit-trick)
            # NOTE: splitting one group's exp between SE and DVE halves
            # (to rebalance 9.5/6.5 chunks) SLOWED every full DVE exp
            # 1285->1520ns - concurrent SE+DVE reads of the same st
            # pool's PSUM banks contend. Do not retry.
            EXP_BIAS = float((15 << 10) + 0.5 - np.log2(1.0397) * 1024.0)
            i16 = mybir.dt.int16
            MULT = mybir.AluOpType.mult
            ADD = mybir.AluOpType.add

            def emit_exp(u):
                p, t, gi, G, c0 = u
                st_cur = qk_fifo.pop(0)
                ex = exp_pool.tile([128, 2 * LT], f16, tag="ex")
                last_tile = (p == NP - 1 and t == NLT - 1)
                if gi in DVE_GROUPS or (last_tile and gi == 6):
                    nc.vector.tensor_scalar(
                        out=ex[:, 0:G * LT].bitcast(i16),
                        in0=st_cur[:, 0:G * LT],
                        scalar1=a2_all[:, p:p + 1], scalar2=EXP_BIAS,
                        op0=MULT, op1=ADD)
                else:
                    nc.scalar.activation(ex[:, 0:G * LT], st_cur[:, 0:G * LT],
                                         EXP, scale=a_all[:, p:p + 1])
                return ex

            emit_loads(0)
            qk_fifo = [emit_qk(units[0], 0), emit_qk(units[1], 1),
                       emit_qk(units[2], 2)]
            # Units are processed in SUPERSLOTS of two so the PE program
            # order per superslot is [QK(i+3), QK(i+4), PV(i), PV(i+1)]:
            # adjacent same-mode matmuls halve the packed<->unpacked array
            # reconfigurations, whose exposed LDWEIGHTS cost ~90ns each
            # (measured: QK->PV / PV->QK issue intervals were ~300ns vs
            # 225ns for PV->PV).
            # Tails are emitted DELAYED and in TWO STAGES: recip+
            # broadcast at the next tile's g5 slot, mul+store two units
            # later. A one-shot tail put the mul in the DVE FIFO right
            # behind its own GpSimd broadcast, so the mul's ~1.7us
            # cross-engine latency head-of-line-blocked the next DVE
            # exp, whose lookahead-QK WAR then stalled the PE 1.3-2.2us
            # at every pair boundary (measured). Two DVE exps get queued
            # between the stages, hiding the broadcast round-trip.
            pending_recips = {}
            pending_muls = {}

            def run_pending(j, split=False):
                if j in pending_recips:
                    u, ot_tile = pending_recips.pop(j)
                    rb = emit_tail_recip(u, ot_tile)
                    # +4: the mul queues on DVE after the NEXT tile's g1
                    # exp, so (a) its ~2us recip->broadcast cross-engine
                    # latency is already paid when it reaches the DVE
                    # FIFO head, and (b) it cannot delay that exp, whose
                    # lookahead-QK WAR otherwise stalls the PE.
                    pending_muls[j + 4] = (u, ot_tile, rb)
                if j in pending_muls:
                    emit_tail_mul(*pending_muls.pop(j), split=split)

            ex_live = {}
            for i in range(0, len(units), 2):
                uA, uB = units[i], units[i + 1]
                ex_live[i] = emit_exp(uA)
                ex_live[i + 1] = emit_exp(uB)
                run_pending(i)
                run_pending(i + 1)
                # Prefetch the next pair's loads ~12 units before first
                # use (the serial gpsimd casting queue needs ~6.5us per
                # pair; 3 units of lead stalled the PE 1-3.7us at every
                # pair boundary - measured).
                if i + 12 < len(units) and units[i + 12][0] != units[i + 11][0]:
                    emit_loads(units[i + 12][0])
                if i in deferred_casts:
                    deferred_casts.pop(i)()
                if i + 3 < len(units):
                    qk_fifo.append(emit_qk(units[i + 3], i + 3))
                if i + 4 < len(units):
                    qk_fifo.append(emit_qk(units[i + 4], i + 4))
                # PVs run DELAYED by one superslot (PV(i-2), PV(i-1)):
                # the ot-buffer WAR on the tail mul and the exp->PV RAW
                # both gain two unit-slots of slack.
                if i - 2 >= 0:
                    for j in (i - 2, i - 1):
                        u = units[j]
                        emit_pv(u, ex_live.pop(j))
                        if u[4] + u[3] == NSC:  # last group of this l-tile
                            pending_recips[j + 6] = (u, cur_ot[0])
            n = len(units)
            for j in (n - 2, n - 1):
                u = units[j]
                emit_pv(u, ex_live.pop(j))
                if u[4] + u[3] == NSC:
                    pending_recips[j + 6] = (u, cur_ot[0])
            while pending_recips or pending_muls:
                j = min(set(pending_recips) | set(pending_muls))
                run_pending(j, split=True)
    nc.compile()
    return nc


def _get_program():
    global _PROGRAM
    if _PROGRAM is None:
        _PROGRAM = _build_program()
    return _PROGRAM


def kernel(queries, keys, values, attn_mask=None, tau=None, delta=None):
    from concourse.bass_utils import run_bass_kernel_spmd

    queries = np.ascontiguousarray(np.asarray(queries, dtype=np.float32))
    keys = np.ascontiguousarray(np.asarray(keys, dtype=np.float32))
    values = np.ascontiguousarray(np.asarray(values, dtype=np.float32))
    tau_flat = np.asarray(tau, dtype=np.float32).reshape(B)

    # pair = b*H + h; per-pair transposed layouts (host does layout only)
    qT_base = queries.transpose(0, 2, 3, 1).reshape(B * H, E, L)
    qT_all = np.ascontiguousarray(
        np.concatenate([qT_base, qT_base], axis=1))  # [32, 128, L] duplicated
    kT_base = keys.transpose(0, 2, 3, 1).reshape(B * H, E, S)
    kc = kT_base.reshape(B * H, E, S // 128, 128)
    kT_all = np.ascontiguousarray(
        np.concatenate([kc[:, :, 0::2, :], kc[:, :, 1::2, :]], axis=1))
    # kT_all: [32, 128, 8, 128]; rows 0:64 = even chunks, 64:128 = odd
    v_all = np.ascontiguousarray(
        values.transpose(0, 2, 1, 3).reshape(B * H, S, E))

    nc = _get_program()
    in_maps = []
    for c in range(NCORES):
        lo = c * NP
        tau_pairs = np.ascontiguousarray(
            tau_flat[[(lo + i) // H for i in range(NP)]].reshape(1, NP))
        in_maps.append({
            "qt": qT_all[lo:lo + NP],
            "kt": kT_all[lo:lo + NP],
            "v": v_all[lo:lo + NP],
            "taus": tau_pairs,
        })

    kwargs = {}
    if os.environ.get("ATTN_TRACE"):
        kwargs["trace"] = True
        tmpdir = os.environ.get("ATTN_TRACE_DIR")
        if tmpdir:
            os.makedirs(tmpdir, exist_ok=True)
            kwargs["tmpdir"] = tmpdir

    res = run_bass_kernel_spmd(nc, in_maps, list(range(NCORES)), **kwargs)
    global LAST_RESULTS
    LAST_RESULTS = res

    o_all = np.concatenate([r["o"] for r in res.results], axis=0)  # [32, E, L]
    out = o_all.astype(np.float32).reshape(B, H, E, L).transpose(0, 3, 1, 2)
    return np.ascontiguousarray(out)  # [B, L, H, E] f32



# revision 42
# speedup vs baseline: 1.0065x; 1.0065x over previous
"""Multi-head attention kernel for Trainium2 (Bass/Tile), 8-core SPMD.

Problem: B=4, L=S=2048, H=8, E=D=64, fp32.
  scores = einsum('blhe,bshe->bhls', Q, K) * tau[b] + delta[b]
  A = softmax(scores / sqrt(E), axis=-1)
  out = einsum('bhls,bshd->blhd', A, V)

Key observations:
  - softmax(a*x + c) == softmax(a*x): the per-batch delta bias cancels.
  - attn_mask is all-False / unused by the reference.
  - B*H = 32 (b,h) pairs, each an independent L x S attention block.
    Shard 4 pairs per core across 8 cores; no cross-core comms.

Per-core kernel design (per (b,h) pair), all matmul operands fp16,
fp32 PSUM:
  - Scores are computed TRANSPOSED: ST[s, l] chunks of [128, 512] so that
    the PV matmul can consume exp(ST) directly as the moving operand with
    full K=128 contraction (no P transposes).
  - QK row-packed: two K=64 matmuls run concurrently in PE row groups
    (0,0)/(64,0); host supplies K^T chunk pairs split across partition
    halves and Q^T duplicated on both halves.
  - exp: 16 chunks/l-tile split 10 on ScalarE (activation Exp, per-batch
    tau/sqrt(E) folded into the activation scale, ~598ns/chunk incl the
    ~335ns fixed cost) + 6 on DVE (bit-trick, ~642ns/chunk), so the two
    exp engines run at ~6.0us/tile next to the PE's ~6.25us/tile.
    (Splitting ONE group's exp between SE and DVE halves to fine-tune
    the balance SLOWED every full DVE exp 1285->1520ns - concurrent
    SE+DVE reads of the same st pool's PSUM banks contend; do not
    retry.)
  - PV: lhsT = V' chunk [s=128, 128] with columns 0..63 all-ones and V
    in columns 64..127, rhs = exp chunk [128, 512], accumulated over 16
    s-chunks into O^T [128, 512] PSUM. The 64 ones columns replicate
    the softmax denominator across PSUM partitions 0..63 at zero PE
    cost. V at partition base 64 because >32-partition DVE accesses
    must start at partition 0 or 64 (verifier-enforced). (PSUM allows
    only ONE live accumulation chain per bank, so the l-sub-split
    "operand-swapped" PV variant with 4 interleaved chains in one bank
    silently corrupts - measured, do not retry.)
  - Tail, two stages: (1) reciprocal_approx_fast straight on PSUM
    partitions 0..63 (custom DVE ops need base_partition 0 - measured)
    - no partition broadcast needed thanks to the replicated
    denominator; (2) TWO units later, multiply O^T rows 64..127 out of
    PSUM (fp16 out) and store. The stage gap keeps the mul's
    cross-engine wait from head-of-line-blocking the next DVE exp,
    whose lookahead-QK WAR otherwise stalls the PE 1.3-2.2us per pair
    (measured).
  - Emission: units processed in SUPERSLOTS of two with LOOKAHEAD 3 and
    PV DELAY 2 - PE program order per superslot is [QK(i+3), QK(i+4),
    PV(i-2), PV(i-1)]. Adjacent same-mode matmuls halve the packed<->
    unpacked PE reconfigurations (~90ns exposed LDWEIGHTS each), and
    the PV delay gives the exp->PV RAW and the tail-mul ot WAR two
    unit-slots of slack. 8 uniform 2-chunk groups per l-tile cycle
    through 3 st pools keyed on the GLOBAL unit index mod 3 (strict
    rotation across tile boundaries). exp_pool MUST be 6-deep: with
    4 bufs, exp(i) WARs the delayed PV(i-4) and the whole pipeline
    slows ~0.6us/tile (measured).
  - Output is stored fp16 (DVE mul writes f16; host upcasts) - halves
    store traffic; adds ~2e-4 rel err vs the 2e-2 budget.

Host side only reshapes/slices/transposes/concats constant ones and
upcasts the fp16 output (sharding + layout); all math is on-device.
Measured: 148.4us (v0 baseline) -> 125.9us.
"""

import os
import numpy as np

B, L, S, H, E = 4, 2048, 2048, 8, 64
NCORES = 8
NP = (B * H) // NCORES  # pairs per core = 4

LT = 512          # l-tile size (columns of ST chunks / PV moving dim)
NLT = L // LT     # 4
NSC = S // 128    # 16 s-chunks
# 8 uniform groups of 2 s-chunks per l-tile, cycling through THREE
# 2-bank st PSUM pools with LOOKAHEAD 3 (trace showed the 2-pool /
# lookahead-2 scheme stalled the PE ~2us/tile on the QK(u+2) WAR
# against exp(u); a 3-deep rotation gives ~2.2us of pipeline slack vs
# the ~1.3us exp latency). stA+stB+stC (2 banks each) + O^T (2 bufs x
# 1 bank) = 8 PSUM banks.
GROUPS = (2,) * 8

_PROGRAM = None
LAST_RESULTS = None  # test harness reads exec_time_ns / trace path from here


def _build_program():
    import concourse.bass as bass
    import concourse.bacc as bacc
    import concourse.tile as tile
    from concourse import mybir

    f32 = mybir.dt.float32
    f16 = mybir.dt.float16
    EXP = mybir.ActivationFunctionType.Exp

    nc = bacc.Bacc("TRN2", target_bir_lowering=False, debug=False,
                   num_devices=NCORES)
    # qt2: Q^T duplicated on both partition halves [128, L].
    # kt2: K^T s-chunk pairs split across partition halves:
    #   kt2[0:64, j, :] = K^T chunk 2j, kt2[64:128, j, :] = chunk 2j+1.
    qT = nc.dram_tensor("qt", [NP, 2 * E, L], f32, kind="ExternalInput").ap()
    kT = nc.dram_tensor("kt", [NP, 2 * E, NSC // 2, 128], f32,
                        kind="ExternalInput").ap()
    v = nc.dram_tensor("v", [NP, S, E], f32, kind="ExternalInput").ap()
    taus = nc.dram_tensor("taus", [1, NP], f32, kind="ExternalInput").ap()
    o = nc.dram_tensor("o", [NP, E, L], f16, kind="ExternalOutput").ap()

    with tile.TileContext(nc) as tc:
        from contextlib import ExitStack
        with ExitStack() as ctx:
            consts = ctx.enter_context(tc.tile_pool(name="consts", bufs=1))
            kq_pool = ctx.enter_context(tc.tile_pool(name="kq", bufs=2))
            v_pool = ctx.enter_context(tc.tile_pool(name="vp", bufs=2))
            exp_pool = ctx.enter_context(tc.tile_pool(name="expp", bufs=7))
            tail_pool = ctx.enter_context(tc.tile_pool(name="tail", bufs=3))
            stA_pool = ctx.enter_context(
                tc.tile_pool(name="stA", bufs=1, space="PSUM"))
            stB_pool = ctx.enter_context(
                tc.tile_pool(name="stB", bufs=1, space="PSUM"))
            stC_pool = ctx.enter_context(
                tc.tile_pool(name="stC", bufs=1, space="PSUM"))
            ot_pool = ctx.enter_context(
                tc.tile_pool(name="ot", bufs=2, space="PSUM"))
            st_pools = (stA_pool, stB_pool, stC_pool)

            # tau[b] per pair, broadcast to all partitions; fold in 1/sqrt(E)
            tau_bc = consts.tile([128, NP], f32)
            nc.sync.dma_start(out=tau_bc, in_=taus.to_broadcast([128, NP]))
            a_all = consts.tile([128, NP], f32)
            nc.scalar.mul(a_all, tau_bc, 1.0 / np.sqrt(float(E)))
            # DVE bit-trick exp scale: tau/sqrt(E) * log2(e) * 2^10 (fp16
            # exponent scale). z = st*a2 + BIAS truncated to int16 and
            # bitcast to fp16 gives 2^(st*a1*log2e) with piecewise-linear
            # mantissa; BIAS folds the fp16 exponent offset (15<<10), +0.5
            # round-to-nearest, and a /1.0397 centering of the one-sided
            # interpolation error (keeps DVE chunks consistent with the
            # exact ScalarE chunks in the softmax mixture).
            a2_all = consts.tile([128, NP], f32)
            nc.scalar.mul(a2_all, tau_bc,
                          np.log2(np.e) * 1024.0 / np.sqrt(float(E)))

            # unit = one exp group: (pair, l-tile, group idx, size, chunk0)
            units = []
            for p in range(NP):
                for t in range(NLT):
                    c0 = 0
                    for gi, G in enumerate(GROUPS):
                        units.append((p, t, gi, G, c0))
                        c0 += G

            pair_tiles = {}
            deferred_casts = {}

            def emit_loads(p):
                # fp16 operands; gpsimd DMA casts f32 -> f16 in flight.
                # Split into first-use-first pieces so the first dependent
                # QK/PV can start long before the whole pair has landed.
                # Pair 0 is latency-critical (nothing hides it): its kt/qt
                # head chunks ride the TWO hardware DGE queues (sync +
                # scalar) as raw f32 in parallel with the gpsimd software
                # casting queue, then DVE (idle at startup) casts them.
                kt_sb = kq_pool.tile([128, NSC // 2, 128], f16, tag="kt")
                qt_sb = kq_pool.tile([128, L], f16, tag="qt")
                vp_sb = v_pool.tile([128, NSC, 128], f16, tag="vp")
                # V' = [ones | zeros x63 | V]: the ones column puts the
                # PV denominator row on PSUM partition 0, where
                # reciprocal_approx_fast can read it directly (custom
                # DVE ops need base_partition 0) - kills v0's 1-lane
                # denominator COPY. O^T rows land at partitions 64..127
                # so the tail's 64-partition DVE ops stay quadrant-
                # aligned (>32-wide DVE accesses must start at 0 or 64).
                # Ones fill ALL of V' columns 0..63 (GpSimd queue):
                # the PV matmul then replicates the softmax denominator
                # across PSUM partitions 0..63 at zero PE cost, so the
                # tail reciprocal runs directly on [64, LT] (same DVE
                # column count as [1, LT]) and the GpSimd
                # partition_broadcast round-trip disappears. For pair 0
                # the memset is emitted between the v-DMAs (gpsimd issue
                # slots are ~1us; the first v packets must start ASAP).
                vre = v[p].rearrange("(n q) e -> q n e", q=128)
                nc.gpsimd.memset(vp_sb[:, :, 0:64], 1.0)
                if p == 0:
                    # Pair 0 is latency-critical: spread it over all 3
                    # DMA queues (each ~45-67GB/s), f32-staged on the hw
                    # queues with DVE casts (DVE is idle until the first
                    # exp). qt tile 0 is host-DUPLICATED data: DMA only
                    # the unique [64, LT] half and replicate via two DVE
                    # casts - halves the first-QK critical DMA. kt24/
                    # kt48 land later, so their casts are emitted
                    # DEFERRED at superslot 2/4 tops (before that slot's
                    # exps - a cast emitted after an exp whose QK needs
                    # it would deadlock the DVE FIFO).
                    stage_kt = consts.tile([128, NSC // 2, 128], f32)
                    stage_qh = consts.tile([64, LT], f32)
                    nc.scalar.dma_start(out=stage_qh, in_=qT[p][0:64, 0:LT])
                    nc.sync.dma_start(out=stage_kt[:, 0:2, :],
                                      in_=kT[p][:, 0:2, :])
                    nc.scalar.dma_start(out=stage_kt[:, 4:8, :],
                                        in_=kT[p][:, 4:8, :])
                    nc.sync.dma_start(out=stage_kt[:, 2:4, :],
                                      in_=kT[p][:, 2:4, :])
                    nc.vector.tensor_copy(qt_sb[0:64, 0:LT], stage_qh)
                    nc.vector.tensor_copy(qt_sb[64:128, 0:LT], stage_qh)
                    nc.vector.tensor_copy(kt_sb[:, 0:2, :],
                                          stage_kt[:, 0:2, :])
                    # kt24 feeds QK(2) which is PRIMED, so its cast must
                    # be emitted here (before the prime). Only kt48's
                    # cast can defer - to superslot 0, emitted BEFORE
                    # the QK(4) append that reads it.
                    nc.vector.tensor_copy(kt_sb[:, 2:4, :],
                                          stage_kt[:, 2:4, :])
                    deferred_casts[0] = lambda: nc.vector.tensor_copy(
                        kt_sb[:, 4:8, :], stage_kt[:, 4:8, :])
                    nc.gpsimd.dma_start(out=vp_sb[:, 0:8, 64:64 + E],
                                        in_=vre[:, 0:8, :])
                    nc.gpsimd.dma_start(out=vp_sb[:, 8:16, 64:64 + E],
                                        in_=vre[:, 8:16, :])
                    for t in range(1, NLT):
                        nc.gpsimd.dma_start(
                            out=qt_sb[:, t * LT:(t + 1) * LT],
                            in_=qT[p][:, t * LT:(t + 1) * LT])
                else:
                    nc.gpsimd.dma_start(out=kt_sb[:, 0:2, :],
                                        in_=kT[p][:, 0:2, :])
                    nc.gpsimd.dma_start(out=qt_sb[:, 0:LT],
                                        in_=qT[p][:, 0:LT])
                    nc.gpsimd.dma_start(out=kt_sb[:, 2:8, :],
                                        in_=kT[p][:, 2:8, :])
                    nc.gpsimd.dma_start(out=vp_sb[:, 0:8, 64:64 + E],
                                        in_=vre[:, 0:8, :])
                    for t in range(1, NLT):
                        nc.gpsimd.dma_start(
                            out=qt_sb[:, t * LT:(t + 1) * LT],
                            in_=qT[p][:, t * LT:(t + 1) * LT])
                    nc.gpsimd.dma_start(out=vp_sb[:, 8:16, 64:64 + E],
                                        in_=vre[:, 8:16, :])
                pair_tiles[p] = (kt_sb, qt_sb, vp_sb)

            def emit_qk(u, ui):
                p, t, gi, G, c0 = u
                kt_sb, qt_sb, _ = pair_tiles[p]
                st_ps = st_pools[ui % 3].tile([128, 2 * LT], f32)
                j = c0 // 2  # packed chunk-pair index
                nc.tensor.matmul(
                    st_ps[:, 0:LT],
                    lhsT=kt_sb[0:64, j, :],
                    rhs=qt_sb[0:64, t * LT:(t + 1) * LT],
                    start=True, stop=True, tile_position=(0, 0))
                nc.tensor.matmul(
                    st_ps[:, LT:2 * LT],
                    lhsT=kt_sb[64:128, j, :],
                    rhs=qt_sb[64:128, t * LT:(t + 1) * LT],
                    start=True, stop=True, tile_position=(64, 0))
                return st_ps

            cur_ot = [None]

            def emit_pv(u, ex):
                p, t, gi, G, c0 = u
                vp_sb = pair_tiles[p][2]
                if c0 == 0:
                    cur_ot[0] = ot_pool.tile([128, LT], f32, name="ot_ps",
                                             tag="ot_ps")
                for k in range(G):
                    c = c0 + k
                    nc.tensor.matmul(
                        cur_ot[0],
                        lhsT=vp_sb[:, c, :],
                        rhs=ex[:, k * LT:(k + 1) * LT],
                        start=(c == 0), stop=(c == NSC - 1))

            def emit_tail_recip(u, ot_tile):
                p, t, gi, G, c0 = u
                # Denominator sits replicated on PSUM partitions 0..63
                # (ones columns of V'): fast-recip straight from PSUM.
                rb = tail_pool.tile([64, LT], f32, tag="rb")
                nc.vector.reciprocal_approx_fast(out=rb,
                                                 in_=ot_tile[0:64, :])
                return rb

            def emit_tail_mul(u, ot_tile, rb, split=False):
                p, t, gi, G, c0 = u
                # Normalize reading O^T (rows 64..127) straight from
                # PSUM; split=True halves the chain across the two hw
                # DMA queues (used for the drain-critical final tiles).
                on = tail_pool.tile([64, LT], f16, tag="on")
                if split:
                    nc.vector.tensor_mul(on[:, 0:LT // 2],
                                         ot_tile[64:64 + E, 0:LT // 2],
                                         rb[:, 0:LT // 2])
                    nc.sync.dma_start(
                        out=o[p, :, t * LT:t * LT + LT // 2],
                        in_=on[:, 0:LT // 2])
                    nc.vector.tensor_mul(on[:, LT // 2:LT],
                                         ot_tile[64:64 + E, LT // 2:LT],
                                         rb[:, LT // 2:LT])
                    nc.scalar.dma_start(
                        out=o[p, :, t * LT + LT // 2:(t + 1) * LT],
                        in_=on[:, LT // 2:LT])
                else:
                    nc.vector.tensor_mul(on, ot_tile[64:64 + E, :], rb)
                    nc.sync.dma_start(out=o[p, :, t * LT:(t + 1) * LT],
                                      in_=on)

            # Software-pipelined emission, lookahead 3: the PE program order
            # per iteration is [QK(u+3), PV(u)], so when PV(u) blocks on
            # exp(u), QK(u+1..3) are already past it, and the WAR of
            # QK(u+3) on exp(u) (same st pool, 3-deep rotation) has ~3
            # group-slots of PE work to hide the exp latency.
            DVE_GROUPS = (1, 3, 5)  # groups exp'd on Vector (bit-trick)
            # NOTE: splitting one group's exp between SE and DVE halves
            # (to rebalance 9.5/6.5 chunks) SLOWED every full DVE exp
            # 1285->1520ns - concurrent SE+DVE reads of the same st
            # pool's PSUM banks contend. Do not retry.
            EXP_BIAS = float((15 << 10) + 0.5 - np.log2(1.0397) * 1024.0)
            i16 = mybir.dt.int16
            MULT = mybir.AluOpType.mult
            ADD = mybir.AluOpType.add

            def emit_exp(u):
                p, t, gi, G, c0 = u
                st_cur = qk_fifo.pop(0)
                ex = exp_pool.tile([128, 2 * LT], f16, tag="ex")
                last_tile = (p == NP - 1 and t == NLT - 1)
                if gi in DVE_GROUPS or (last_tile and gi == 6):
                    nc.vector.tensor_scalar(
                        out=ex[:, 0:G * LT].bitcast(i16),
                        in0=st_cur[:, 0:G * LT],
                        scalar1=a2_all[:, p:p + 1], scalar2=EXP_BIAS,
                        op0=MULT, op1=ADD)
                else:
                    nc.scalar.activation(ex[:, 0:G * LT], st_cur[:, 0:G * LT],
                                         EXP, scale=a_all[:, p:p + 1])
                return ex

            emit_loads(0)
            qk_fifo = [emit_qk(units[0], 0), emit_qk(units[1], 1),
                       emit_qk(units[2], 2)]
            # Units are processed in SUPERSLOTS of two so the PE program
            # order per superslot is [QK(i+3), QK(i+4), PV(i), PV(i+1)]:
            # adjacent same-mode matmuls halve the packed<->unpacked array
            # reconfigurations, whose exposed LDWEIGHTS cost ~90ns each
            # (measured: QK->PV / PV->QK issue intervals were ~300ns vs
            # 225ns for PV->PV).
            # Tails are emitted DELAYED and in TWO STAGES: recip+
            # broadcast at the next tile's g5 slot, mul+store two units
            # later. A one-shot tail put the mul in the DVE FIFO right
            # behind its own GpSimd broadcast, so the mul's ~1.7us
            # cross-engine latency head-of-line-blocked the next DVE
            # exp, whose lookahead-QK WAR then stalled the PE 1.3-2.2us
            # at every pair boundary (measured). Two DVE exps get queued
            # between the stages, hiding the broadcast round-trip.
            pending_recips = {}
            pending_muls = {}

            def run_pending(j, split=False):
                if j in pending_recips:
                    u, ot_tile = pending_recips.pop(j)
                    rb = emit_tail_recip(u, ot_tile)
                    # +4: the mul queues on DVE after the NEXT tile's g1
                    # exp, so (a) its ~2us recip->broadcast cross-engine
                    # latency is already paid when it reaches the DVE
                    # FIFO head, and (b) it cannot delay that exp, whose
                    # lookahead-QK WAR otherwise stalls the PE.
                    pending_muls[j + 4] = (u, ot_tile, rb)
                if j in pending_muls:
                    emit_tail_mul(*pending_muls.pop(j), split=split)

            ex_live = {}
            for i in range(0, len(units), 2):
                uA, uB = units[i], units[i + 1]
                ex_live[i] = emit_exp(uA)
                ex_live[i + 1] = emit_exp(uB)
                run_pending(i)
                run_pending(i + 1)
                # Prefetch the next pair's loads ~12 units before first
                # use (the serial gpsimd casting queue needs ~6.5us per
                # pair; 3 units of lead stalled the PE 1-3.7us at every
                # pair boundary - measured).
                if i + 12 < len(units) and units[i + 12][0] != units[i + 11][0]:
                    emit_loads(units[i + 12][0])
                if i in deferred_casts:
                    deferred_casts.pop(i)()
                if i + 3 < len(units):
                    qk_fifo.append(emit_qk(units[i + 3], i + 3))
                if i + 4 < len(units):
                    qk_fifo.append(emit_qk(units[i + 4], i + 4))
                # PVs run DELAYED by one superslot (PV(i-2), PV(i-1)):
                # the ot-buffer WAR on the tail mul and the exp->PV RAW
                # both gain two unit-slots of slack.
                if i - 2 >= 0:
                    for j in (i - 2, i - 1):
                        u = units[j]
                        emit_pv(u, ex_live.pop(j))
                        if u[4] + u[3] == NSC:  # last group of this l-tile
                            pending_recips[j + 6] = (u, cur_ot[0])
            n = len(units)
            for j in (n - 2, n - 1):
                u = units[j]
                emit_pv(u, ex_live.pop(j))
                if u[4] + u[3] == NSC:
                    pending_recips[j + 6] = (u, cur_ot[0])
            while pending_recips or pending_muls:
                j = min(set(pending_recips) | set(pending_muls))
                run_pending(j, split=True)
    nc.compile()
    return nc


def _get_program():
    global _PROGRAM
    if _PROGRAM is None:
        _PROGRAM = _build_program()
    return _PROGRAM


def kernel(queries, keys, values, attn_mask=None, tau=None, delta=None):
    from concourse.bass_utils import run_bass_kernel_spmd

    queries = np.ascontiguousarray(np.asarray(queries, dtype=np.float32))
    keys = np.ascontiguousarray(np.asarray(keys, dtype=np.float32))
    values = np.ascontiguousarray(np.asarray(values, dtype=np.float32))
    tau_flat = np.asarray(tau, dtype=np.float32).reshape(B)

    # pair = b*H + h; per-pair transposed layouts (host does layout only)
    qT_base = queries.transpose(0, 2, 3, 1).reshape(B * H, E, L)
    qT_all = np.ascontiguousarray(
        np.concatenate([qT_base, qT_base], axis=1))  # [32, 128, L] duplicated
    kT_base = keys.transpose(0, 2, 3, 1).reshape(B * H, E, S)
    kc = kT_base.reshape(B * H, E, S // 128, 128)
    kT_all = np.ascontiguousarray(
        np.concatenate([kc[:, :, 0::2, :], kc[:, :, 1::2, :]], axis=1))
    # kT_all: [32, 128, 8, 128]; rows 0:64 = even chunks, 64:128 = odd
    v_all = np.ascontiguousarray(
        values.transpose(0, 2, 1, 3).reshape(B * H, S, E))

    nc = _get_program()
    in_maps = []
    for c in range(NCORES):
        lo = c * NP
        tau_pairs = np.ascontiguousarray(
            tau_flat[[(lo + i) // H for i in range(NP)]].reshape(1, NP))
        in_maps.append({
            "qt": qT_all[lo:lo + NP],
            "kt": kT_all[lo:lo + NP],
            "v": v_all[lo:lo + NP],
            "taus": tau_pairs,
        })

    kwargs = {}
    if os.environ.get("ATTN_TRACE"):
        kwargs["trace"] = True
        tmpdir = os.environ.get("ATTN_TRACE_DIR")
        if tmpdir:
            os.makedirs(tmpdir, exist_ok=True)
            kwargs["tmpdir"] = tmpdir

    res = run_bass_kernel_spmd(nc, in_maps, list(range(NCORES)), **kwargs)
    global LAST_RESULTS
    LAST_RESULTS = res

    o_all = np.concatenate([r["o"] for r in res.results], axis=0)  # [32, E, L]
    out = o_all.astype(np.float32).reshape(B, H, E, L).transpose(0, 3, 1, 2)
    return np.ascontiguousarray(out)  # [B, L, H, E] f32

